# revision 8
# baseline (speedup 1.0000x reference)
"""MAB (multihead attention block) Trainium2 kernel, v3.

Sharding: 8 cores = 4 batches x 2 query-halves (same as baseline).

v3 changes vs baseline:
  - bf16 inputs/weights/activations (psum stays f32): halves DMA + SBUF.
  - Superblock streaming: key loop split into 2 superblocks of 8 chunks;
    A@V partials accumulate in PSUM per superblock and are added into an
    SBUF accumulator, so attention (and the exp stream on ACT) starts after
    only the first quarter of the projections.
  - Projections interleaved into the attention combo stream.
  - Softmax division: reciprocal broadcast via a tiny PE matmul (ones x rec)
    instead of a DRAM DMA round-trip.
  - Output projection per query-chunk for epilogue overlap; residual (qt+bv)
    precomputed on the Pool engine; relu on ACT.
  - PSUM: logits pool 3 deep (decouples PE from ACT), A@V accumulators split
    into even/odd single-bank tiles.

Layouts (per core):
  qT, kT  feature-major [512, Nq/Nk] bf16 (heads = 64-row blocks)
  v       token-major   [Nk, 772] bf16   (4 even-head blocks [v(64)|mask],
                                          4 odd-head blocks [mask|0*63|v(64)])
  logits  S^T [keys, queries]; exp on ACT; denominators accumulate through
          the mask columns of v during the A@V matmul.
"""

import math
import os

import ml_dtypes
import numpy as np

import concourse.bass as bass
import concourse.tile as tile
from concourse import bacc, mybir

F32 = mybir.dt.float32
BF16 = mybir.dt.bfloat16
MM = mybir.dt.float32r

DIM = 512
NQ = 1024  # queries per core
NK = 2048  # keys per core
H = 8
D = 64
P = 128
FCH = DIM // P  # 4 feature chunks
KD = DIM // P  # 4 contraction chunks
TCH = NK // P  # 16 token/key chunks
QCH = NQ // 512  # 2 query chunks of 512
SCALE = 1.0 / math.sqrt(DIM)

VW = 4 * 65 + 4 * 128  # 772
EVEN_OFF = [65 * i for i in range(4)]
ODD_OFF = [260 + 128 * i for i in range(4)]
SB = [(0, 8), (8, 16)]  # superblocks of key chunks

INPUT_SPECS = {
    "QT": ((DIM, NQ), BF16),
    "VT": ((DIM, NK), BF16),
    "WqT": ((DIM, DIM), BF16),
    "WkT": ((DIM, DIM), BF16),
    "WvTp": ((DIM, VW), BF16),
    "WoT": ((DIM, DIM), BF16),
    "bq": ((DIM,), F32),
    "bk": ((DIM,), F32),
    "bv": ((DIM,), F32),
    "bo": ((DIM,), F32),
    "mask01": ((NK,), F32),
    "maskrep": ((NK, 4), F32),
}
OUTPUT_SPEC = ((DIM, NQ), F32)


def _r(ap):
    return ap.bitcast(MM)


def emit(ctx, tc, io):
    nc = tc.nc
    AF = mybir.ActivationFunctionType
    OP = mybir.AluOpType

    consts = ctx.enter_context(tc.tile_pool(name="consts", bufs=1))
    bigs = ctx.enter_context(tc.tile_pool(name="bigs", bufs=1))
    att = ctx.enter_context(tc.tile_pool(name="att", bufs=9))
    sm = ctx.enter_context(tc.tile_pool(name="sm", bufs=6))
    ps_s = ctx.enter_context(tc.tile_pool(name="ps_s", bufs=3, space="PSUM"))
    ps_n = ctx.enter_context(tc.tile_pool(name="ps_n", bufs=1, space="PSUM"))

    # ---- weights / inputs; DMA queue order = emission order ---------------
    # wk + first V chunk lead so the k-projection starts ASAP.
    wk_sb = consts.tile([P, KD, DIM], BF16)
    wk_src = io["WkT"].rearrange("(kd p) f -> p kd f", p=P)
    vtin = bigs.tile([P, KD, NK], BF16)
    vt_src = io["VT"].rearrange("(kd p) t -> p kd t", p=P)
    nc.sync.dma_start(wk_sb[:, 0:2, :], wk_src[:, 0:2, :])
    nc.sync.dma_start(vtin[:, 0:2, 0:512], vt_src[:, 0:2, 0:512])
    nc.sync.dma_start(wk_sb[:, 2:4, :], wk_src[:, 2:4, :])
    nc.sync.dma_start(vtin[:, 2:4, 0:512], vt_src[:, 2:4, 0:512])
    nc.sync.dma_start(vtin[:, :, 512:1024], vt_src[:, :, 512:1024])
    wq_sb = consts.tile([P, KD, DIM], BF16)
    nc.sync.dma_start(wq_sb, io["WqT"].rearrange("(kd p) f -> p kd f", p=P))

    # ---- small constants --------------------------------------------------
    bq_sb = consts.tile([P, FCH], F32)
    nc.sync.dma_start(bq_sb, io["bq"].rearrange("(c p) -> p c", p=P))
    bk_sb = consts.tile([P, FCH], F32)
    nc.sync.dma_start(bk_sb, io["bk"].rearrange("(c p) -> p c", p=P))
    bv_sb = consts.tile([P, FCH], F32)
    nc.sync.dma_start(bv_sb, io["bv"].rearrange("(c p) -> p c", p=P))
    bo_sb = consts.tile([P, FCH], F32)
    nc.sync.dma_start(bo_sb, io["bo"].rearrange("(c p) -> p c", p=P))
    mask_sb = consts.tile([P, TCH], F32)
    nc.sync.dma_start(mask_sb, io["mask01"].rearrange("(c p) -> p c", p=P))
    mrep_sb = consts.tile([P, TCH, 4], F32)
    nc.sync.dma_start(mrep_sb, io["maskrep"].rearrange("(c p) r -> p c r", p=P))
    ones_sb = consts.tile([P, D], BF16)
    nc.vector.memset(ones_sb, 1.0)

    # warm the ACT exp table early so the table load overlaps DMA
    warm = consts.tile([1, 1], F32)
    nc.vector.memset(warm, 0.0)
    nc.scalar.activation(warm, warm, AF.Exp)
    qtin = bigs.tile([P, KD, NQ], BF16)
    qt_src = io["QT"].rearrange("(kd p) t -> p kd t", p=P)
    nc.sync.dma_start(qtin[:, :, 0:512], qt_src[:, :, 0:512])
    wvp_sb = consts.tile([P, KD, VW], BF16)
    nc.sync.dma_start(wvp_sb, io["WvTp"].rearrange("(kd p) f -> p kd f", p=P))
    nc.sync.dma_start(qtin[:, :, 512:1024], qt_src[:, :, 512:1024])
    nc.sync.dma_start(vtin[:, :, 1024:1536], vt_src[:, :, 1024:1536])
    nc.sync.dma_start(vtin[:, :, 1536:2048], vt_src[:, :, 1536:2048])
    wo_sb = consts.tile([P, KD, DIM], BF16)
    nc.sync.dma_start(wo_sb, io["WoT"].rearrange("(kd p) f -> p kd f", p=P))

    # ---- persistent results ----------------------------------------------
    v_sb = bigs.tile([P, TCH, VW], BF16)
    qt_sb = bigs.tile([P, FCH, NQ], BF16)
    kt_sb = bigs.tile([P, FCH, NK], BF16)
    ot_sb = bigs.tile([P, FCH, NQ], BF16)
    acc = bigs.tile([P, 8, 1024], F32)  # per-combo numerators; c = qc*4+pr
    qbv_sb = bigs.tile([P, 8, 512], BF16)  # qt + bv residual, precomputed on Pool

    def qbv_pre(pr, qc):
        c = qc * 4 + pr
        nc.gpsimd.tensor_scalar_add(
            qbv_sb[:, c, :], qt_sb[:, pr, qc * 512:(qc + 1) * 512],
            bv_sb[:, pr:pr + 1],
        )

    def kproj(n):
        for j in range(2):
            ps = ps_s.tile([P, 1024], F32, tag="s")
            for h in range(2):
                fc = 2 * j + h
                for kd in range(KD):
                    nc.tensor.matmul(
                        ps[:, h * 512:(h + 1) * 512],
                        wk_sb[:, kd, fc * P:(fc + 1) * P],
                        vtin[:, kd, n * 512:(n + 1) * 512],
                        start=(kd == 0), stop=(kd == KD - 1),
                    )
            for h in range(2):
                fc = 2 * j + h
                nc.vector.tensor_scalar_add(
                    kt_sb[:, fc, n * 512:(n + 1) * 512],
                    ps[:, h * 512:(h + 1) * 512],
                    bk_sb[:, fc:fc + 1],
                )

    def qproj(qc):
        for j in range(2):
            ps = ps_s.tile([P, 1024], F32, tag="s")
            for h in range(2):
                fc = 2 * j + h
                for kd in range(KD):
                    nc.tensor.matmul(
                        ps[:, h * 512:(h + 1) * 512],
                        wq_sb[:, kd, fc * P:(fc + 1) * P],
                        qtin[:, kd, qc * 512:(qc + 1) * 512],
                        start=(kd == 0), stop=(kd == KD - 1),
                    )
            for h in range(2):
                fc = 2 * j + h
                nc.vector.tensor_scalar_add(
                    qt_sb[:, fc, qc * 512:(qc + 1) * 512],
                    ps[:, h * 512:(h + 1) * 512],
                    bq_sb[:, fc:fc + 1],
                )

    def vproj_a(t):
        ps = ps_s.tile([P, 1024], F32, tag="s", name=f"psv_{t}")
        for kd in range(KD):
            nc.tensor.matmul(
                ps[:, 0:512], vtin[:, kd, t * P:(t + 1) * P], wvp_sb[:, kd, 0:512],
                start=(kd == 0), stop=(kd == KD - 1),
            )
        return ps

    def vproj_b(t, ps):
        for kd in range(KD):
            nc.tensor.matmul(
                ps[:, 512:VW], vtin[:, kd, t * P:(t + 1) * P], wvp_sb[:, kd, 512:VW],
                start=(kd == 0), stop=(kd == KD - 1),
            )
        # zero masked tokens (rows); mask cols are 0 here
        nc.vector.tensor_scalar_mul(v_sb[:, t, :], ps[:, 0:VW], mask_sb[:, t:t + 1])
        even_cols = v_sb[:, t, 0:260].rearrange("p (e c) -> p e c", c=65)[:, :, 64]
        nc.vector.tensor_copy(even_cols, mrep_sb[:, t, :])
        odd_cols = v_sb[:, t, 260:VW].rearrange("p (o c) -> p o c", c=128)[:, :, 0]
        nc.vector.tensor_copy(odd_cols, mrep_sb[:, t, :])

    def vproj(t):
        vproj_b(t, vproj_a(t))

    def lg_exp(pr, qc, kc):
        s_ps = ps_s.tile([P, 1024], F32, tag="s")
        for hh in range(2):
            nc.tensor.matmul(
                s_ps[:, hh * 512:(hh + 1) * 512],
                kt_sb[64 * hh:64 * hh + 64, pr, kc * P:(kc + 1) * P],
                qt_sb[64 * hh:64 * hh + 64, pr, qc * 512:(qc + 1) * 512],
                start=True, stop=True,
                tile_position=(64 * hh, 0),
            )
        es = att.tile([P, 1024], BF16, tag="es")
        nc.scalar.activation(es, s_ps, AF.Exp, scale=SCALE)
        return es

    def av(pr, np_ps, es, kc, first, last):
        np_e, np_o = np_ps
        nc.tensor.matmul(
            np_e[0:65, :],
            v_sb[:, kc, EVEN_OFF[pr]:EVEN_OFF[pr] + 65],
            es[:, 0:512],
            start=first, stop=last,
        )
        nc.tensor.matmul(
            np_o,
            v_sb[:, kc, ODD_OFF[pr]:ODD_OFF[pr] + 128],
            es[:, 512:1024],
            start=first, stop=last,
        )

    def acc_update(c, np_ps, sbi):
        np_e, np_o = np_ps
        if sbi == 0:
            nc.vector.tensor_copy(acc[0:65, c, 0:512], np_e[0:65, :])
            nc.vector.tensor_copy(acc[:, c, 512:1024], np_o)
        else:
            nc.vector.tensor_tensor(
                acc[0:65, c, 0:512], acc[0:65, c, 0:512], np_e[0:65, :],
                op=OP.add,
            )
            nc.vector.tensor_tensor(
                acc[:, c, 512:1024], acc[:, c, 512:1024], np_o,
                op=OP.add,
            )

    def tail_recip(pr, qc):
        c = qc * 4 + pr
        rec = sm.tile([P, 1024], BF16, tag="rec")
        with nc.allow_low_precision(reason="bf16 reciprocal for PE broadcast"):
            nc.vector.reciprocal(rec[64:65, 0:512], acc[64:65, c, 0:512])
            nc.vector.reciprocal(rec[0:1, 512:1024], acc[0:1, c, 512:1024])
        return rec

    def tail_bc(rec, pool=None):
        # reciprocal row broadcast via bf16 K=1 PE matmuls (ones x rec)
        bc = ps_s.tile([P, 1024], F32, tag="s", name="bc")
        nc.tensor.matmul(
            bc[0:64, 0:512], ones_sb[64:65, 0:64], rec[64:65, 0:512],
            start=True, stop=True,
        )
        nc.tensor.matmul(
            bc[64:128, 512:1024], ones_sb[0:1, 0:64], rec[0:1, 512:1024],
            start=True, stop=True,
        )
        return bc

    def tail_t1(pr, qc, bc, last=False):
        c = qc * 4 + pr
        for hh in range(2):
            psl = slice(0, 64) if hh == 0 else slice(64, 128)
            fsl = slice(0, 512) if hh == 0 else slice(512, 1024)
            t1 = sm.tile([P, 512], F32, tag="t1")
            nc.vector.tensor_tensor(
                t1[psl, :], acc[psl, c, fsl], bc[psl, fsl], op=OP.mult
            )
            eng = nc.gpsimd if (not last or hh == 0) else nc.vector
            eng.tensor_tensor(
                ot_sb[psl, pr, qc * 512:(qc + 1) * 512],
                t1[psl, :],
                qbv_sb[psl, c, :],
                op=OP.add,
            )

    out_dst = io["outT"].rearrange("(fc p) q -> p fc q", p=P)

    def oproj_pre(qc, j, n_ifc=FCH):
        ups = ps_s.tile([P, 1024], F32, tag="s", name=f"ups_{qc}_{j}")
        for h in range(2):
            ofc = 2 * j + h
            for ifc in range(n_ifc):
                nc.tensor.matmul(
                    ups[:, h * 512:(h + 1) * 512],
                    wo_sb[:, ifc, ofc * P:(ofc + 1) * P],
                    ot_sb[:, ifc, qc * 512:(qc + 1) * 512],
                    start=(ifc == 0), stop=(ifc == FCH - 1),
                )
        return ups

    def oproj_fin(qc, j, ups, s_ifc=FCH, fin_eng=None):
        for h in range(2):
            ofc = 2 * j + h
            for ifc in range(s_ifc, FCH):
                nc.tensor.matmul(
                    ups[:, h * 512:(h + 1) * 512],
                    wo_sb[:, ifc, ofc * P:(ofc + 1) * P],
                    ot_sb[:, ifc, qc * 512:(qc + 1) * 512],
                    start=False, stop=(ifc == FCH - 1),
                )
        for h in range(2):
            ofc = 2 * j + h
            r1 = sm.tile([P, 512], F32, tag="r1")
            # relu+bias on ACT keeps it off the jammed DVE queue
            nc.scalar.activation(
                r1, ups[:, h * 512:(h + 1) * 512], AF.Relu,
                bias=bo_sb[:, ofc:ofc + 1],
            )
            fin = sm.tile([P, 512], F32, tag="fin")
            eng = (nc.vector if h == 0 else nc.gpsimd) if fin_eng else nc.gpsimd
            eng.tensor_tensor(
                fin, r1, ot_sb[:, ofc, qc * 512:(qc + 1) * 512], op=OP.add
            )
            nc.sync.dma_start(out_dst[:, ofc, qc * 512:(qc + 1) * 512], fin)

    def oproj_g(qc, j):
        oproj_fin(qc, j, oproj_pre(qc, j))

    # ---- main emission: one software-pipelined stream ---------------------
    # Pre-stream projections (needed by the first attention steps).
    kproj(0)
    kproj(1)
    qproj(0)
    for t in range(8):
        vproj(t)

    # All (combo, key-chunk) steps of both superblocks, pipelined with A@V
    # lagging logits/exp by DEPTH so it never head-of-line blocks the PE
    # queue; remaining projections and tails are spliced in as inserts.
    DEPTH = 6
    combos = [(pr, qc) for qc in range(2) for pr in range(4)]
    steps = []
    for sbi in range(2):
        k0, k1 = SB[sbi]
        for ci, (pr, qc) in enumerate(combos):
            for kc in range(k0, k1):
                steps.append((sbi, ci, pr, qc, kc))

    from collections import defaultdict as _dd

    inserts = _dd(list)
    # qproj(1) needed before step 32 (first qc1 logits)
    inserts[6].append(lambda: qproj(1))
    # residual qt+bv precomputes on Pool (idle early)
    for i in range(4):
        inserts[1 + 2 * i].append(lambda pr=i: qbv_pre(pr, 0))
        inserts[8 + 2 * i].append(lambda pr=i: qbv_pre(pr, 1))
    # kproj(2)/(3) needed before step 64 (first sb1 logits, kc8/kc12)
    inserts[14].append(lambda: kproj(2))
    inserts[26].append(lambda: kproj(3))
    # vproj(8..15) needed before sb1 A@V of the matching chunk (step 66+)
    for i, t in enumerate(range(8, 16)):
        inserts[34 + 4 * i].append(lambda t=t: vproj(t))
    # oproj(0) after tail(3, 0)'s deferred t1 lands (step 105)
    inserts[108].append(lambda: oproj_g(0, 0))
    inserts[112].append(lambda: oproj_g(0, 1))

    np_t = {}
    es_q = {}
    deferred = _dd(list)
    total = len(steps)
    for j in range(total + DEPTH):
        for fn in inserts.pop(j, []):
            fn()
        for fn in deferred.pop(j, []):
            fn()
        if j < total:
            sbi, ci, pr, qc, kc = steps[j]
            if kc == SB[sbi][0]:
                np_t[sbi, ci] = (
                    ps_n.tile([65, 512], F32, tag="ne", name=f"npe_{sbi}_{ci}"),
                    ps_n.tile([P, 512], F32, tag="no", name=f"npo_{sbi}_{ci}"),
                )
            es_q[j] = lg_exp(pr, qc, kc)
        if j >= DEPTH:
            sbi, ci, pr, qc, kc = steps[j - DEPTH]
            k0, k1 = SB[sbi]
            av(pr, np_t[sbi, ci], es_q.pop(j - DEPTH), kc, kc == k0, kc == k1 - 1)
            if kc == k1 - 1:
                np_ps = np_t.pop((sbi, ci))
                acc_update(qc * 4 + pr, np_ps, sbi)
                if sbi == 1:
                    # tail chain deferred so each stage arrives at its queue
                    # with dependencies already satisfied (no head-of-line)
                    state = {}
                    if j - DEPTH == total - 1:
                        last_state = state  # handled in the end sequence
                        state["rec"] = tail_recip(pr, qc)
                    else:
                        deferred[j + 2].append(
                            lambda state=state, pr=pr, qc=qc: state.__setitem__(
                                "rec", tail_recip(pr, qc))
                        )
                        deferred[j + 4].append(
                            lambda state=state: state.__setitem__(
                                "bc", tail_bc(state["rec"]))
                        )
                        deferred[j + 6].append(
                            lambda state=state, pr=pr, qc=qc: tail_t1(
                                pr, qc, state["bc"])
                        )
    for jj in sorted(deferred):
        for fn in deferred[jj]:
            fn()
    # End sequence: overlap oproj(1)'s ifc 0-2 partial sums with the last
    # combo's tail chain, then finish with the short ifc-3 dependency.
    ups0 = oproj_pre(1, 0, n_ifc=3)
    ups1 = oproj_pre(1, 1, n_ifc=3)
    bc = tail_bc(last_state["rec"])
    tail_t1(3, 1, bc, last=True)
    oproj_fin(1, 0, ups0, s_ifc=3, fin_eng=nc.vector)
    oproj_fin(1, 1, ups1, s_ifc=3, fin_eng=nc.vector)


def make_core_inputs(Q, V, mask, Wq, bq, Wk, bk, Wv, bv, Wo, bo, core):
    b, s = divmod(core, 2)
    f32 = np.float32
    bf16 = ml_dtypes.bfloat16
    QT = np.ascontiguousarray(Q[b, s * NQ:(s + 1) * NQ, :].T).astype(bf16)
    VT = np.ascontiguousarray(V[b].T).astype(bf16)
    WvT = np.asarray(Wv).T.astype(f32)
    WvTp = np.zeros((DIM, VW), dtype=f32)
    for i in range(4):  # even heads 2i: [v(64) | mask]
        WvTp[:, EVEN_OFF[i]:EVEN_OFF[i] + 64] = WvT[:, (2 * i) * 64:(2 * i + 1) * 64]
    for i in range(4):  # odd heads 2i+1: [mask | 0*63 | v(64)]
        WvTp[:, ODD_OFF[i] + 64:ODD_OFF[i] + 128] = WvT[:, (2 * i + 1) * 64:(2 * i + 2) * 64]
    m01 = np.asarray(mask[b]).astype(f32)
    return {
        "QT": QT,
        "VT": VT,
        "WqT": np.ascontiguousarray(np.asarray(Wq).T).astype(bf16),
        "WkT": np.ascontiguousarray(np.asarray(Wk).T).astype(bf16),
        "WvTp": WvTp.astype(bf16),
        "WoT": np.ascontiguousarray(np.asarray(Wo).T).astype(bf16),
        "bq": np.asarray(bq, dtype=f32),
        "bk": np.asarray(bk, dtype=f32),
        "bv": np.asarray(bv, dtype=f32),
        "bo": np.asarray(bo, dtype=f32),
        "mask01": m01,
        "maskrep": np.ascontiguousarray(np.repeat(m01[:, None], 4, axis=1)),
    }


_CACHE = {}


def build_program():
    if "nc" in _CACHE:
        return _CACHE["nc"]
    from contextlib import ExitStack

    nc = bacc.Bacc("TRN2", target_bir_lowering=False, debug=False)
    io = {}
    for name, (shape, dt) in INPUT_SPECS.items():
        io[name] = nc.dram_tensor(name, list(shape), dt, kind="ExternalInput").ap()
    io["outT"] = nc.dram_tensor("outT", list(OUTPUT_SPEC[0]), OUTPUT_SPEC[1],
                                kind="ExternalOutput").ap()
    with tile.TileContext(nc) as tc:
        with ExitStack() as ctx:
            emit(ctx, tc, io)
    nc.compile()
    _CACHE["nc"] = nc
    return nc


def kernel(Q, V, mask, Wq, bq, Wk, bk, Wv, bv, Wo, bo):
    from concourse.bass_utils import run_bass_kernel_spmd

    nc = build_program()
    args = (Q, V, mask, Wq, bq, Wk, bk, Wv, bv, Wo, bo)
    in_maps = [make_core_inputs(*args, core=c) for c in range(8)]
    res = run_bass_kernel_spmd(
        nc, in_maps, core_ids=list(range(8)),
        trace=bool(int(os.environ.get("KTRACE", "0"))),
    )
    _CACHE["last_result"] = res
    B = 4
    out = np.empty((B, 2 * NQ, DIM), np.float32)
    for c in range(8):
        b, s = divmod(c, 2)
        out[b, s * NQ:(s + 1) * NQ, :] = res.results[c]["outT"].T
    return out


# revision 10
# speedup vs baseline: 3.6638x; 3.6638x over previous
"""MAB (multihead attention block) Trainium2 kernel, v3.

Sharding: 8 cores = 4 batches x 2 query-halves (same as baseline).

v3 changes vs baseline:
  - bf16 inputs/weights/activations (psum stays f32): halves DMA + SBUF.
  - Superblock streaming: key loop split into 2 superblocks of 8 chunks;
    A@V partials accumulate in PSUM per superblock and are added into an
    SBUF accumulator, so attention (and the exp stream on ACT) starts after
    only the first quarter of the projections.
  - Projections interleaved into the attention combo stream.
  - Softmax division: reciprocal broadcast via a tiny PE matmul (ones x rec)
    instead of a DRAM DMA round-trip.
  - Output projection per query-chunk for epilogue overlap; residual (qt+bv)
    precomputed on the Pool engine; relu on ACT.
  - PSUM: logits pool 3 deep (decouples PE from ACT), A@V accumulators split
    into even/odd single-bank tiles.

Layouts (per core):
  qT, kT  feature-major [512, Nq/Nk] bf16 (heads = 64-row blocks)
  v       token-major   [Nk, 772] bf16   (4 even-head blocks [v(64)|mask],
                                          4 odd-head blocks [mask|0*63|v(64)])
  logits  S^T [keys, queries]; exp on ACT; denominators accumulate through
          the mask columns of v during the A@V matmul.
"""

import math
import os

import ml_dtypes
import numpy as np

import concourse.bass as bass
import concourse.tile as tile
from concourse import bacc, mybir

F32 = mybir.dt.float32
BF16 = mybir.dt.bfloat16
MM = mybir.dt.float32r

DIM = 512
NQ = 1024  # queries per core
NK = 2048  # keys per core
H = 8
D = 64
P = 128
FCH = DIM // P  # 4 feature chunks
KD = DIM // P  # 4 contraction chunks
TCH = NK // P  # 16 token/key chunks
QCH = NQ // 512  # 2 query chunks of 512
SCALE = 1.0 / math.sqrt(DIM)

VW = 4 * 65 + 4 * 128  # 772
EVEN_OFF = [65 * i for i in range(4)]
ODD_OFF = [260 + 128 * i for i in range(4)]
SB = [(0, 8), (8, 16)]  # superblocks of key chunks

INPUT_SPECS = {
    "QT": ((DIM, NQ), BF16),
    "VT": ((DIM, NK), BF16),
    "WqT": ((DIM, DIM), BF16),
    "WkT": ((DIM, DIM), BF16),
    "WvTp": ((DIM, VW), BF16),
    "WoT": ((DIM, DIM), BF16),
    "bq": ((DIM,), F32),
    "bk": ((DIM,), F32),
    "bv": ((DIM,), F32),
    "bo": ((DIM,), F32),
    "mask01": ((NK,), F32),
    "maskrep": ((NK, 4), F32),
}
OUTPUT_SPEC = ((DIM, NQ), F32)


def _r(ap):
    return ap.bitcast(MM)


def emit(ctx, tc, io):
    nc = tc.nc
    AF = mybir.ActivationFunctionType
    OP = mybir.AluOpType

    consts = ctx.enter_context(tc.tile_pool(name="consts", bufs=1))
    bigs = ctx.enter_context(tc.tile_pool(name="bigs", bufs=1))
    att = ctx.enter_context(tc.tile_pool(name="att", bufs=13))
    sm = ctx.enter_context(tc.tile_pool(name="sm", bufs=4))
    ps_s = ctx.enter_context(tc.tile_pool(name="ps_s", bufs=3, space="PSUM"))
    ps_n = ctx.enter_context(tc.tile_pool(name="ps_n", bufs=1, space="PSUM"))

    # ---- weights / inputs; DMA queue order = emission order ---------------
    # wk + first V chunk lead so the k-projection starts ASAP.
    wk_sb = consts.tile([P, KD, DIM], BF16)
    wk_src = io["WkT"].rearrange("(kd p) f -> p kd f", p=P)
    vtin = bigs.tile([P, KD, NK], BF16)
    vt_src = io["VT"].rearrange("(kd p) t -> p kd t", p=P)
    nc.sync.dma_start(wk_sb[:, 0:1, :], wk_src[:, 0:1, :])
    nc.sync.dma_start(vtin[:, 0:1, 0:512], vt_src[:, 0:1, 0:512])
    nc.sync.dma_start(wk_sb[:, 1:2, :], wk_src[:, 1:2, :])
    nc.sync.dma_start(vtin[:, 1:2, 0:512], vt_src[:, 1:2, 0:512])
    nc.sync.dma_start(wk_sb[:, 2:4, :], wk_src[:, 2:4, :])
    nc.sync.dma_start(vtin[:, 2:4, 0:512], vt_src[:, 2:4, 0:512])
    nc.sync.dma_start(vtin[:, :, 512:1024], vt_src[:, :, 512:1024])
    wq_sb = consts.tile([P, KD, DIM], BF16)
    nc.sync.dma_start(wq_sb, io["WqT"].rearrange("(kd p) f -> p kd f", p=P))

    # ---- small constants --------------------------------------------------
    bq_sb = consts.tile([P, FCH], F32)
    nc.sync.dma_start(bq_sb, io["bq"].rearrange("(c p) -> p c", p=P))
    bk_sb = consts.tile([P, FCH], F32)
    nc.sync.dma_start(bk_sb, io["bk"].rearrange("(c p) -> p c", p=P))
    bv_sb = consts.tile([P, FCH], F32)
    nc.sync.dma_start(bv_sb, io["bv"].rearrange("(c p) -> p c", p=P))
    bo_sb = consts.tile([P, FCH], F32)
    nc.sync.dma_start(bo_sb, io["bo"].rearrange("(c p) -> p c", p=P))
    mask_sb = consts.tile([P, TCH], F32)
    nc.sync.dma_start(mask_sb, io["mask01"].rearrange("(c p) -> p c", p=P))
    mrep_sb = consts.tile([P, TCH, 4], F32)
    nc.sync.dma_start(mrep_sb, io["maskrep"].rearrange("(c p) r -> p c r", p=P))
    ones_sb = consts.tile([P, D], BF16)
    nc.vector.memset(ones_sb, 1.0)

    # warm the ACT exp table early so the table load overlaps DMA
    warm = consts.tile([1, 1], F32)
    nc.vector.memset(warm, 0.0)
    nc.scalar.activation(warm, warm, AF.Exp)
    qtin = bigs.tile([P, KD, NQ], BF16)
    qt_src = io["QT"].rearrange("(kd p) t -> p kd t", p=P)
    nc.sync.dma_start(qtin[:, :, 0:512], qt_src[:, :, 0:512])
    wvp_sb = consts.tile([P, KD, VW], BF16)
    nc.sync.dma_start(wvp_sb, io["WvTp"].rearrange("(kd p) f -> p kd f", p=P))
    nc.sync.dma_start(qtin[:, :, 512:1024], qt_src[:, :, 512:1024])
    nc.sync.dma_start(vtin[:, :, 1024:1536], vt_src[:, :, 1024:1536])
    nc.sync.dma_start(vtin[:, :, 1536:2048], vt_src[:, :, 1536:2048])
    wo_sb = consts.tile([P, KD, DIM], BF16)
    nc.sync.dma_start(wo_sb, io["WoT"].rearrange("(kd p) f -> p kd f", p=P))

    # ---- persistent results ----------------------------------------------
    v_sb = bigs.tile([P, TCH, VW], BF16)
    qt_sb = bigs.tile([P, FCH, NQ], BF16)
    kt_sb = bigs.tile([P, FCH, NK], BF16)
    ot_sb = bigs.tile([P, FCH, NQ], BF16)
    acc = bigs.tile([P, 8, 1024], F32)  # per-combo numerators; c = qc*4+pr
    qbv_sb = bigs.tile([P, 8, 512], BF16)  # qt + bv residual, precomputed on Pool

    def qbv_pre(pr, qc):
        c = qc * 4 + pr
        nc.gpsimd.tensor_scalar_add(
            qbv_sb[:, c, :], qt_sb[:, pr, qc * 512:(qc + 1) * 512],
            bv_sb[:, pr:pr + 1],
        )

    def kproj(n):
        for j in range(2):
            ps = ps_s.tile([P, 1024], F32, tag="s")
            for h in range(2):
                fc = 2 * j + h
                for kd in range(KD):
                    nc.tensor.matmul(
                        ps[:, h * 512:(h + 1) * 512],
                        wk_sb[:, kd, fc * P:(fc + 1) * P],
                        vtin[:, kd, n * 512:(n + 1) * 512],
                        start=(kd == 0), stop=(kd == KD - 1),
                    )
            for h in range(2):
                fc = 2 * j + h
                nc.vector.tensor_scalar_add(
                    kt_sb[:, fc, n * 512:(n + 1) * 512],
                    ps[:, h * 512:(h + 1) * 512],
                    bk_sb[:, fc:fc + 1],
                )

    def qproj(qc):
        for j in range(2):
            ps = ps_s.tile([P, 1024], F32, tag="s")
            for h in range(2):
                fc = 2 * j + h
                for kd in range(KD):
                    nc.tensor.matmul(
                        ps[:, h * 512:(h + 1) * 512],
                        wq_sb[:, kd, fc * P:(fc + 1) * P],
                        qtin[:, kd, qc * 512:(qc + 1) * 512],
                        start=(kd == 0), stop=(kd == KD - 1),
                    )
            for h in range(2):
                fc = 2 * j + h
                nc.vector.tensor_scalar_add(
                    qt_sb[:, fc, qc * 512:(qc + 1) * 512],
                    ps[:, h * 512:(h + 1) * 512],
                    bq_sb[:, fc:fc + 1],
                )

    def vproj_a(t):
        ps = ps_s.tile([P, 1024], F32, tag="s", name=f"psv_{t}")
        for kd in range(KD):
            nc.tensor.matmul(
                ps[:, 0:512], vtin[:, kd, t * P:(t + 1) * P], wvp_sb[:, kd, 0:512],
                start=(kd == 0), stop=(kd == KD - 1),
            )
        return ps

    def vproj_b(t, ps):
        for kd in range(KD):
            nc.tensor.matmul(
                ps[:, 512:VW], vtin[:, kd, t * P:(t + 1) * P], wvp_sb[:, kd, 512:VW],
                start=(kd == 0), stop=(kd == KD - 1),
            )
        # zero masked tokens (rows); mask cols are 0 here
        nc.vector.tensor_scalar_mul(v_sb[:, t, :], ps[:, 0:VW], mask_sb[:, t:t + 1])
        even_cols = v_sb[:, t, 0:260].rearrange("p (e c) -> p e c", c=65)[:, :, 64]
        nc.vector.tensor_copy(even_cols, mrep_sb[:, t, :])
        odd_cols = v_sb[:, t, 260:VW].rearrange("p (o c) -> p o c", c=128)[:, :, 0]
        nc.vector.tensor_copy(odd_cols, mrep_sb[:, t, :])

    def vproj(t):
        vproj_b(t, vproj_a(t))

    def lg_exp(pr, qc, kc):
        s_ps = ps_s.tile([P, 1024], F32, tag="s")
        for hh in range(2):
            nc.tensor.matmul(
                s_ps[:, hh * 512:(hh + 1) * 512],
                kt_sb[64 * hh:64 * hh + 64, pr, kc * P:(kc + 1) * P],
                qt_sb[64 * hh:64 * hh + 64, pr, qc * 512:(qc + 1) * 512],
                start=True, stop=True,
                tile_position=(64 * hh, 0),
            )
        es = att.tile([P, 1024], BF16, tag="es")
        nc.scalar.activation(es, s_ps, AF.Exp, scale=SCALE)
        return es

    def av(pr, np_ps, es, kc, first, last):
        np_e, np_o = np_ps
        nc.tensor.matmul(
            np_e[0:65, :],
            v_sb[:, kc, EVEN_OFF[pr]:EVEN_OFF[pr] + 65],
            es[:, 0:512],
            start=first, stop=last,
        )
        nc.tensor.matmul(
            np_o,
            v_sb[:, kc, ODD_OFF[pr]:ODD_OFF[pr] + 128],
            es[:, 512:1024],
            start=first, stop=last,
        )

    def acc_update(c, np_ps, sbi):
        np_e, np_o = np_ps
        if sbi == 0:
            nc.vector.tensor_copy(acc[0:65, c, 0:512], np_e[0:65, :])
            nc.vector.tensor_copy(acc[:, c, 512:1024], np_o)
        else:
            nc.vector.tensor_tensor(
                acc[0:65, c, 0:512], acc[0:65, c, 0:512], np_e[0:65, :],
                op=OP.add,
            )
            nc.vector.tensor_tensor(
                acc[:, c, 512:1024], acc[:, c, 512:1024], np_o,
                op=OP.add,
            )

    def tail_recip(pr, qc):
        c = qc * 4 + pr
        rec = sm.tile([P, 1024], BF16, tag="rec")
        with nc.allow_low_precision(reason="bf16 reciprocal for PE broadcast"):
            nc.vector.reciprocal(rec[64:65, 0:512], acc[64:65, c, 0:512])
            nc.vector.reciprocal(rec[0:1, 512:1024], acc[0:1, c, 512:1024])
        return rec

    def tail_bc(rec, pool=None):
        # reciprocal row broadcast via bf16 K=1 PE matmuls (ones x rec)
        bc = ps_s.tile([P, 1024], F32, tag="s", name="bc")
        nc.tensor.matmul(
            bc[0:64, 0:512], ones_sb[64:65, 0:64], rec[64:65, 0:512],
            start=True, stop=True,
        )
        nc.tensor.matmul(
            bc[64:128, 512:1024], ones_sb[0:1, 0:64], rec[0:1, 512:1024],
            start=True, stop=True,
        )
        return bc

    def tail_t1(pr, qc, bc, last=False):
        c = qc * 4 + pr
        for hh in range(2):
            psl = slice(0, 64) if hh == 0 else slice(64, 128)
            fsl = slice(0, 512) if hh == 0 else slice(512, 1024)
            t1 = sm.tile([P, 512], F32, tag="t1")
            nc.vector.tensor_tensor(
                t1[psl, :], acc[psl, c, fsl], bc[psl, fsl], op=OP.mult
            )
            eng = nc.gpsimd if (not last or hh == 0) else nc.vector
            eng.tensor_tensor(
                ot_sb[psl, pr, qc * 512:(qc + 1) * 512],
                t1[psl, :],
                qbv_sb[psl, c, :],
                op=OP.add,
            )

    out_dst = io["outT"].rearrange("(fc p) q -> p fc q", p=P)

    def oproj_pre(qc, j, n_ifc=FCH):
        ups = ps_s.tile([P, 1024], F32, tag="s", name=f"ups_{qc}_{j}")
        for h in range(2):
            ofc = 2 * j + h
            for ifc in range(n_ifc):
                nc.tensor.matmul(
                    ups[:, h * 512:(h + 1) * 512],
                    wo_sb[:, ifc, ofc * P:(ofc + 1) * P],
                    ot_sb[:, ifc, qc * 512:(qc + 1) * 512],
                    start=(ifc == 0), stop=(ifc == FCH - 1),
                )
        return ups

    def oproj_fin(qc, j, ups, s_ifc=FCH, fin_eng=None):
        for h in range(2):
            ofc = 2 * j + h
            for ifc in range(s_ifc, FCH):
                nc.tensor.matmul(
                    ups[:, h * 512:(h + 1) * 512],
                    wo_sb[:, ifc, ofc * P:(ofc + 1) * P],
                    ot_sb[:, ifc, qc * 512:(qc + 1) * 512],
                    start=False, stop=(ifc == FCH - 1),
                )
        for h in range(2):
            ofc = 2 * j + h
            r1 = sm.tile([P, 512], F32, tag="r1")
            # relu+bias on ACT keeps it off the jammed DVE queue
            nc.scalar.activation(
                r1, ups[:, h * 512:(h + 1) * 512], AF.Relu,
                bias=bo_sb[:, ofc:ofc + 1],
            )
            fin = sm.tile([P, 512], F32, tag="fin")
            eng = (nc.vector if h == 0 else nc.gpsimd) if fin_eng else nc.gpsimd
            eng.tensor_tensor(
                fin, r1, ot_sb[:, ofc, qc * 512:(qc + 1) * 512], op=OP.add
            )
            nc.sync.dma_start(out_dst[:, ofc, qc * 512:(qc + 1) * 512], fin)

    def oproj_g(qc, j):
        oproj_fin(qc, j, oproj_pre(qc, j))

    # ---- main emission: one software-pipelined stream ---------------------
    # Pre-stream projections (needed by the first attention steps).
    kproj(0)
    kproj(1)
    qproj(0)
    for t in range(8):
        vproj(t)

    # All (combo, key-chunk) steps of both superblocks, pipelined with A@V
    # lagging logits/exp by DEPTH so it never head-of-line blocks the PE
    # queue; remaining projections and tails are spliced in as inserts.
    DEPTH = 10
    combos = [(pr, qc) for qc in range(2) for pr in range(4)]
    steps = []
    for sbi in range(2):
        k0, k1 = SB[sbi]
        for ci, (pr, qc) in enumerate(combos):
            for kc in range(k0, k1):
                steps.append((sbi, ci, pr, qc, kc))

    from collections import defaultdict as _dd

    inserts = _dd(list)
    # qproj(1) needed before step 32 (first qc1 logits)
    inserts[6].append(lambda: qproj(1))
    # residual qt+bv precomputes on Pool (idle early)
    for i in range(4):
        inserts[1 + 2 * i].append(lambda pr=i: qbv_pre(pr, 0))
        inserts[8 + 2 * i].append(lambda pr=i: qbv_pre(pr, 1))
    # kproj(2)/(3) needed before step 64 (first sb1 logits, kc8/kc12)
    inserts[14].append(lambda: kproj(2))
    inserts[26].append(lambda: kproj(3))
    # vproj(8..15) needed before sb1 A@V of the matching chunk (step 66+)
    for i, t in enumerate(range(8, 16)):
        inserts[34 + 4 * i].append(lambda t=t: vproj(t))
    # oproj(0) after tail(3, 0)'s deferred t1 lands (step 105)
    inserts[112].append(lambda: oproj_g(0, 0))
    inserts[116].append(lambda: oproj_g(0, 1))

    np_t = {}
    es_q = {}
    deferred = _dd(list)
    total = len(steps)
    for j in range(total + DEPTH):
        for fn in inserts.pop(j, []):
            fn()
        for fn in deferred.pop(j, []):
            fn()
        if j < total:
            sbi, ci, pr, qc, kc = steps[j]
            if kc == SB[sbi][0]:
                np_t[sbi, ci] = (
                    ps_n.tile([65, 512], F32, tag="ne", name=f"npe_{sbi}_{ci}"),
                    ps_n.tile([P, 512], F32, tag="no", name=f"npo_{sbi}_{ci}"),
                )
            es_q[j] = lg_exp(pr, qc, kc)
        if j >= DEPTH:
            sbi, ci, pr, qc, kc = steps[j - DEPTH]
            k0, k1 = SB[sbi]
            av(pr, np_t[sbi, ci], es_q.pop(j - DEPTH), kc, kc == k0, kc == k1 - 1)
            if kc == k1 - 1:
                np_ps = np_t.pop((sbi, ci))
                acc_update(qc * 4 + pr, np_ps, sbi)
                if sbi == 1:
                    # tail chain deferred so each stage arrives at its queue
                    # with dependencies already satisfied (no head-of-line)
                    state = {}
                    if j - DEPTH == total - 1:
                        last_state = state  # handled in the end sequence
                        state["rec"] = tail_recip(pr, qc)
                    else:
                        deferred[j + 2].append(
                            lambda state=state, pr=pr, qc=qc: state.__setitem__(
                                "rec", tail_recip(pr, qc))
                        )
                        deferred[j + 4].append(
                            lambda state=state: state.__setitem__(
                                "bc", tail_bc(state["rec"]))
                        )
                        deferred[j + 6].append(
                            lambda state=state, pr=pr, qc=qc: tail_t1(
                                pr, qc, state["bc"])
                        )
    for jj in sorted(deferred):
        for fn in deferred[jj]:
            fn()
    # End sequence: overlap oproj(1)'s ifc 0-2 partial sums with the last
    # combo's tail chain, then finish with the short ifc-3 dependency.
    ups0 = oproj_pre(1, 0, n_ifc=3)
    ups1 = oproj_pre(1, 1, n_ifc=3)
    bc = tail_bc(last_state["rec"])
    tail_t1(3, 1, bc, last=True)
    oproj_fin(1, 0, ups0, s_ifc=3, fin_eng=nc.vector)
    oproj_fin(1, 1, ups1, s_ifc=3, fin_eng=nc.vector)


def make_core_inputs(Q, V, mask, Wq, bq, Wk, bk, Wv, bv, Wo, bo, core):
    b, s = divmod(core, 2)
    f32 = np.float32
    bf16 = ml_dtypes.bfloat16
    QT = np.ascontiguousarray(Q[b, s * NQ:(s + 1) * NQ, :].T).astype(bf16)
    VT = np.ascontiguousarray(V[b].T).astype(bf16)
    WvT = np.asarray(Wv).T.astype(f32)
    WvTp = np.zeros((DIM, VW), dtype=f32)
    for i in range(4):  # even heads 2i: [v(64) | mask]
        WvTp[:, EVEN_OFF[i]:EVEN_OFF[i] + 64] = WvT[:, (2 * i) * 64:(2 * i + 1) * 64]
    for i in range(4):  # odd heads 2i+1: [mask | 0*63 | v(64)]
        WvTp[:, ODD_OFF[i] + 64:ODD_OFF[i] + 128] = WvT[:, (2 * i + 1) * 64:(2 * i + 2) * 64]
    m01 = np.asarray(mask[b]).astype(f32)
    return {
        "QT": QT,
        "VT": VT,
        "WqT": np.ascontiguousarray(np.asarray(Wq).T).astype(bf16),
        "WkT": np.ascontiguousarray(np.asarray(Wk).T).astype(bf16),
        "WvTp": WvTp.astype(bf16),
        "WoT": np.ascontiguousarray(np.asarray(Wo).T).astype(bf16),
        "bq": np.asarray(bq, dtype=f32),
        "bk": np.asarray(bk, dtype=f32),
        "bv": np.asarray(bv, dtype=f32),
        "bo": np.asarray(bo, dtype=f32),
        "mask01": m01,
        "maskrep": np.ascontiguousarray(np.repeat(m01[:, None], 4, axis=1)),
    }


_CACHE = {}


def build_program():
    if "nc" in _CACHE:
        return _CACHE["nc"]
    from contextlib import ExitStack

    nc = bacc.Bacc("TRN2", target_bir_lowering=False, debug=False)
    io = {}
    for name, (shape, dt) in INPUT_SPECS.items():
        io[name] = nc.dram_tensor(name, list(shape), dt, kind="ExternalInput").ap()
    io["outT"] = nc.dram_tensor("outT", list(OUTPUT_SPEC[0]), OUTPUT_SPEC[1],
                                kind="ExternalOutput").ap()
    with tile.TileContext(nc) as tc:
        with ExitStack() as ctx:
            emit(ctx, tc, io)
    nc.compile()
    _CACHE["nc"] = nc
    return nc


def kernel(Q, V, mask, Wq, bq, Wk, bk, Wv, bv, Wo, bo):
    from concourse.bass_utils import run_bass_kernel_spmd

    nc = build_program()
    args = (Q, V, mask, Wq, bq, Wk, bk, Wv, bv, Wo, bo)
    in_maps = [make_core_inputs(*args, core=c) for c in range(8)]
    res = run_bass_kernel_spmd(
        nc, in_maps, core_ids=list(range(8)),
        trace=bool(int(os.environ.get("KTRACE", "0"))),
    )
    _CACHE["last_result"] = res
    B = 4
    out = np.empty((B, 2 * NQ, DIM), np.float32)
    for c in range(8):
        b, s = divmod(c, 2)
        out[b, s * NQ:(s + 1) * NQ, :] = res.results[c]["outT"].T
    return out
